# revision 5
# baseline (speedup 1.0000x reference)
"""Trainium2 Bass kernel for the hypergraph-conv survival model.

Graph/data parallel over 8 NeuronCores: core k owns graphs 2k, 2k+1 (a
contiguous node range, since `batch` is sorted).

All message passing runs feature-major ([128 feat partitions, columns])
against one SBUF-resident gather table (z nodes or reduced hyperedges,
phase-alternating):
  - expansion gathers (table columns by incidence) via gpsimd ap_gather
  - segment sums via tensor_tensor_scan (cumsum along the free dim) +
    boundary-position gathers + subtract
  - per-core partial hyperedge tables AllReduce'd in DRAM
The pooled features and the small MLP head run per-core on the two
pooled graph columns. Weights are replicated.
"""

import sys

sys.path.insert(0, "/opt/trn_rl_repo")

import numpy as np

# ---- static problem sizes (from the reference) ----
N = 100_000
E = 800_000
M = 25_000
B_GRAPHS = 16
F_IN = 64
H = 128
EPS = 1e-5
NCORE = 8

GP = 6656               # per-graph node-column stride (13 * 512)
NKP = 2 * GP            # node columns per core (13312)
NZT = NKP + 16          # z gather-table cols (zero pads at NKP..)
M_PAD = 25088           # padded hyperedge count (28 * 896)
NET = M_PAD + 16        # eR gather-table cols (zero pads at M_PAD..)
NSEGA = 896             # hedges per phase-A chunk
NCHA = M_PAD // NSEGA   # 28
NSEGB = 512             # node cols per phase-B chunk
NCHB = NKP // NSEGB     # 26 (graph 0 = chunks 0..12, graph 1 = 13..25)
SCOL = 512              # stage matmul column tile

_COMPILED = None
_CA = None
_CB = None


def _build_nc(CA, CB):
    import concourse.bacc as bacc
    import concourse.mybir as mybir
    from concourse.tile import TileContext
    from concourse import library_config

    f32 = mybir.dt.float32
    i16 = mybir.dt.int16
    ADD = mybir.AluOpType.add
    SUB = mybir.AluOpType.subtract
    MULT = mybir.AluOpType.mult
    MAX = mybir.AluOpType.max
    BYP = mybir.AluOpType.bypass
    AX = mybir.AxisListType.X

    CMAX = max(CA, CB)

    nc = bacc.Bacc("TRN2", target_bir_lowering=False, num_devices=NCORE)

    def inp(name, shape, dt=f32):
        return nc.dram_tensor(name, shape, dt, kind="ExternalInput")

    xT = inp("xT", [F_IN, NKP])
    idxA = inp("idxA", [128, NCHA * CA // 16], i16)
    bidxA = inp("bidxA", [128, NCHA * 2 * NSEGA // 16], i16)
    idxB = inp("idxB", [128, NCHB * CB // 16], i16)
    bidxB = inp("bidxB", [128, NCHB * 2 * NSEGB // 16], i16)
    binv_r = inp("binv_r", [128, M_PAD])
    dinv_r = inp("dinv_r", [128, NKP])
    pc_d = inp("pc", [128, 4])      # pool pad-correction (conv1 g0,g1, conv2 g0,g1)
    psc_d = inp("psc", [128, 2])    # pool 1/n scale (g0, g1)
    W0_d = inp("W0", [F_IN, H])
    Wc1_d = inp("Wc1", [H, H])
    Wc2_d = inp("Wc2", [H, H])
    WgA_d = inp("WgA", [H, H])
    WgB_d = inp("WgB", [H, H])
    W1_d = inp("W1f", [H, 64])
    W2_d = inp("W2f", [64, 32])
    W3_d = inp("W3", [32, 4])
    b0_d = inp("b0c", [H, 1])
    bc1_d = inp("bc1c", [H, 1])
    bc2_d = inp("bc2c", [H, 1])
    bg_d = inp("bgc", [H, 1])
    b1_d = inp("b1c", [64, 1])
    b2_d = inp("b2c", [32, 1])
    out_d = nc.dram_tensor("out", [4, 2], f32, kind="ExternalOutput")

    z2_h = nc.dram_tensor("z2_h", [128, NKP], f32)
    eA1 = nc.dram_tensor("eA1", [128, M_PAD], f32)
    eR1 = nc.dram_tensor("eR1", [128, M_PAD], f32, addr_space="Shared")
    eA2 = nc.dram_tensor("eA2", [128, M_PAD], f32)
    eR2 = nc.dram_tensor("eR2", [128, M_PAD], f32, addr_space="Shared")

    with TileContext(nc) as tc:
        with (
            tc.tile_pool(name="c", bufs=1) as cpool,
            tc.tile_pool(name="tb", bufs=1) as tbpool,
            tc.tile_pool(name="g", bufs=2) as gpool,
            tc.tile_pool(name="bs", bufs=2) as bspool,
            tc.tile_pool(name="st", bufs=2) as stpool,
            tc.tile_pool(name="sc", bufs=2) as scpool,
            tc.tile_pool(name="ix", bufs=2) as ixpool,
            tc.tile_pool(name="ps", bufs=2, space="PSUM") as pspool,
            tc.tile_pool(name="acc", bufs=1) as accpool,
        ):
            nc.gpsimd.load_library(library_config.ap_gather)

            def load_sb(dram, shape, dt=f32):
                t = cpool.tile(shape, dt, tag=dram.name + "_sb")
                nc.sync.dma_start(out=t[:], in_=dram[:, :])
                return t

            W0s = load_sb(W0_d, [F_IN, H])
            Wc1s = load_sb(Wc1_d, [H, H])
            Wc2s = load_sb(Wc2_d, [H, H])
            WgAs = load_sb(WgA_d, [H, H])
            WgBs = load_sb(WgB_d, [H, H])
            W1s = load_sb(W1_d, [H, 64])
            W2s = load_sb(W2_d, [64, 32])
            W3s = load_sb(W3_d, [32, 4])
            b0s = load_sb(b0_d, [H, 1])
            bc1s = load_sb(bc1_d, [H, 1])
            bc2s = load_sb(bc2_d, [H, 1])
            bgs = load_sb(bg_d, [H, 1])
            b1s = load_sb(b1_d, [64, 1])
            b2s = load_sb(b2_d, [32, 1])
            pcs = load_sb(pc_d, [128, 4])
            pscs = load_sb(psc_d, [128, 2])

            # shared gather table: z (NZT cols) in A-phases, eR (NET) in B
            tab = tbpool.tile([128, NET, 1], f32)

            p1acc = accpool.tile([128, 2], f32)
            nc.vector.memset(p1acc[:], 0.0)
            p2acc = accpool.tile([128, 2], f32)
            nc.vector.memset(p2acc[:], 0.0)

            # ---- stage: tab[:, :NKP] = (relu(x@W0+b0)) @ Wc1 ----
            for j in range(NKP // SCOL):
                sl = slice(j * SCOL, (j + 1) * SCOL)
                xc = stpool.tile([F_IN, SCOL], f32, tag="xc")
                nc.sync.dma_start(out=xc[:], in_=xT[:, sl])
                ps1 = pspool.tile([128, SCOL], f32, tag="ps")
                nc.tensor.matmul(ps1[:], W0s[:], xc[:], start=True, stop=True)
                h1 = stpool.tile([128, SCOL], f32, tag="h1")
                nc.vector.tensor_scalar(h1[:], ps1[:], b0s[:], 0.0, ADD, MAX)
                ps2 = pspool.tile([128, SCOL], f32, tag="ps")
                nc.tensor.matmul(ps2[:], Wc1s[:], h1[:], start=True, stop=True)
                nc.scalar.copy(tab[:, sl, 0], ps2[:])
            nc.vector.memset(tab[:, NKP:NZT, 0], 0.0)

            def phase_A(eA_dram):
                for c in range(NCHA):
                    ix = ixpool.tile([128, CA // 16], i16, tag="ixA")
                    nc.sync.dma_start(
                        out=ix[:], in_=idxA[:, c * CA // 16:(c + 1) * CA // 16])
                    bx = ixpool.tile([128, 2 * NSEGA // 16], i16, tag="bxA")
                    nc.sync.dma_start(
                        out=bx[:],
                        in_=bidxA[:, c * 2 * NSEGA // 16:(c + 1) * 2 * NSEGA // 16])
                    bv = scpool.tile([128, NSEGA], f32, tag="bv")
                    nc.sync.dma_start(
                        out=bv[:], in_=binv_r[:, c * NSEGA:(c + 1) * NSEGA])

                    g = gpool.tile([128, CMAX, 1], f32, tag="g")
                    nc.gpsimd.ap_gather(
                        g[:, :CA, :], tab[:, :NZT, :], ix[:],
                        channels=128, num_elems=NZT, d=1, num_idxs=CA)
                    nc.vector.tensor_tensor_scan(
                        g[:, :CA, 0], g[:, :CA, 0], g[:, :CA, 0], 0.0, ADD, BYP)
                    bsel = bspool.tile([128, 2 * NSEGA, 1], f32, tag="bsel")
                    nc.gpsimd.ap_gather(
                        bsel[:], g[:, :CA, :], bx[:],
                        channels=128, num_elems=CA, d=1, num_idxs=2 * NSEGA)
                    wk = bspool.tile([128, NSEGA], f32, tag="wk")
                    nc.vector.tensor_tensor(
                        wk[:], bsel[:, :NSEGA, 0], bsel[:, NSEGA:, 0], SUB)
                    nc.vector.tensor_tensor(wk[:], wk[:], bv[:], MULT)
                    nc.sync.dma_start(
                        out=eA_dram[:, c * NSEGA:(c + 1) * NSEGA], in_=wk[:])

            def phase_B(eR_dram, bias_s, pacc, Wnext, write_z):
                nc.sync.dma_start(out=tab[:, :M_PAD, 0], in_=eR_dram[:, :])
                nc.vector.memset(tab[:, M_PAD:NET, 0], 0.0)
                for c in range(NCHB):
                    ix = ixpool.tile([128, CB // 16], i16, tag="ixB")
                    nc.sync.dma_start(
                        out=ix[:], in_=idxB[:, c * CB // 16:(c + 1) * CB // 16])
                    bx = ixpool.tile([128, 2 * NSEGB // 16], i16, tag="bxB")
                    nc.sync.dma_start(
                        out=bx[:],
                        in_=bidxB[:, c * 2 * NSEGB // 16:(c + 1) * 2 * NSEGB // 16])
                    dv = scpool.tile([128, NSEGB], f32, tag="dv")
                    nc.sync.dma_start(
                        out=dv[:], in_=dinv_r[:, c * NSEGB:(c + 1) * NSEGB])

                    g = gpool.tile([128, CMAX, 1], f32, tag="g")
                    nc.gpsimd.ap_gather(
                        g[:, :CB, :], tab[:, :NET, :], ix[:],
                        channels=128, num_elems=NET, d=1, num_idxs=CB)
                    nc.vector.tensor_tensor_scan(
                        g[:, :CB, 0], g[:, :CB, 0], g[:, :CB, 0], 0.0, ADD, BYP)
                    bsel = bspool.tile([128, 2 * NSEGA, 1], f32, tag="bsel")
                    nc.gpsimd.ap_gather(
                        bsel[:, :2 * NSEGB, :], g[:, :CB, :], bx[:],
                        channels=128, num_elems=CB, d=1, num_idxs=2 * NSEGB)
                    h = bspool.tile([128, NSEGA], f32, tag="wk")
                    nc.vector.tensor_tensor(
                        h[:, :NSEGB], bsel[:, :NSEGB, 0],
                        bsel[:, NSEGB:2 * NSEGB, 0], SUB)
                    nc.vector.tensor_tensor(h[:, :NSEGB], h[:, :NSEGB], dv[:], MULT)
                    nc.vector.tensor_scalar(
                        h[:, :NSEGB], h[:, :NSEGB], bias_s[:], 0.0, ADD, MAX)

                    gi = 0 if c < NCHB // 2 else 1
                    rs = bspool.tile([128, 1], f32, tag="rs")
                    nc.vector.tensor_reduce(rs[:], h[:, :NSEGB], AX, ADD)
                    nc.vector.tensor_add(
                        pacc[:, gi:gi + 1], pacc[:, gi:gi + 1], rs[:])

                    if write_z:
                        zp = pspool.tile([128, NSEGB], f32, tag="ps")
                        nc.tensor.matmul(
                            zp[:], Wnext[:], h[:, :NSEGB], start=True, stop=True)
                        zc = stpool.tile([128, NSEGB], f32, tag="h1")
                        nc.scalar.copy(zc[:], zp[:])
                        nc.sync.dma_start(
                            out=z2_h[:, c * NSEGB:(c + 1) * NSEGB], in_=zc[:])

            phase_A(eA1)
            nc.gpsimd.collective_compute(
                "AllReduce", ADD, replica_groups=[list(range(NCORE))],
                ins=[eA1.ap().opt()], outs=[eR1.ap().opt()])
            phase_B(eR1, bc1s, p1acc, Wc2s, True)
            # reload z2 as the gather table for conv2 phase A
            nc.sync.dma_start(out=tab[:, :NKP, 0], in_=z2_h[:, :])
            nc.vector.memset(tab[:, NKP:NZT, 0], 0.0)
            phase_A(eA2)
            nc.gpsimd.collective_compute(
                "AllReduce", ADD, replica_groups=[list(range(NCORE))],
                ins=[eA2.ap().opt()], outs=[eR2.ap().opt()])
            phase_B(eR2, bc2s, p2acc, None, False)

            # ---- pooled features: p = (pacc - corr) * scale ----
            p1 = accpool.tile([128, 2], f32)
            nc.vector.tensor_tensor(p1[:], p1acc[:], pcs[:, 0:2], SUB)
            nc.vector.tensor_tensor(p1[:], p1[:], pscs[:], MULT)
            p2 = accpool.tile([128, 2], f32)
            nc.vector.tensor_tensor(p2[:], p2acc[:], pcs[:, 2:4], SUB)
            nc.vector.tensor_tensor(p2[:], p2[:], pscs[:], MULT)

            # ---- MLP head on the two pooled columns ----
            gps = pspool.tile([128, 2], f32, tag="mlp")
            nc.tensor.matmul(gps[:], WgAs[:], p1[:], start=True, stop=False)
            nc.tensor.matmul(gps[:], WgBs[:], p2[:], start=False, stop=True)
            gb = accpool.tile([128, 2], f32)
            nc.vector.tensor_scalar(gb[:], gps[:], bgs[:], None, ADD)
            h1ps = pspool.tile([64, 2], f32, tag="mlp")
            nc.tensor.matmul(h1ps[:], W1s[:], gb[:], start=True, stop=True)
            h1m = accpool.tile([64, 2], f32)
            nc.vector.tensor_scalar(h1m[:], h1ps[:], b1s[:], 0.0, ADD, MAX)
            h2ps = pspool.tile([32, 2], f32, tag="mlp")
            nc.tensor.matmul(h2ps[:], W2s[:], h1m[:], start=True, stop=True)
            h2m = accpool.tile([32, 2], f32)
            nc.vector.tensor_scalar(h2m[:], h2ps[:], b2s[:], 0.0, ADD, MAX)
            ops = pspool.tile([4, 2], f32, tag="mlp")
            nc.tensor.matmul(ops[:], W3s[:], h2m[:], start=True, stop=True)
            om = accpool.tile([4, 2], f32)
            nc.vector.tensor_copy(om[:], ops[:])
            nc.sync.dma_start(out=out_d[:, :], in_=om[:])

    nc.compile()
    return nc


def _wrap_idx(idx):
    return np.tile(np.asarray(idx, np.int16).reshape(-1, 16).T, (8, 1)).copy()


def _chunk_lists(keys, vals, nseg, nch, zcol, ca=None):
    """Sorted (keys, vals) -> per-chunk padded slot lists + boundary idxs.

    Chunk c covers segments [c*nseg, (c+1)*nseg). Slot 0 of each chunk is a
    pad (gathers the zero column), then the chunk's values in key order,
    then zero-column pads up to `ca`. Boundary list per chunk: nseg end
    positions then nseg start positions into the chunk's slot space.
    """
    seg_edges = np.searchsorted(keys, np.arange(0, nch * nseg + 1), side="left")
    ch_start = seg_edges[0:nch * nseg:nseg]
    ch_end = seg_edges[nseg::nseg]
    cnt = ch_end - ch_start
    needed = int(cnt.max()) + 1
    if ca is None:
        return None, None, needed
    assert needed <= ca, (needed, ca)
    slots = np.full((nch, ca), zcol, np.int32)
    bnd = np.zeros((nch, 2 * nseg), np.int32)
    for c in range(nch):
        a, b = ch_start[c], ch_end[c]
        n = b - a
        slots[c, 1:1 + n] = vals[a:b]
        loc = seg_edges[c * nseg:(c + 1) * nseg + 1] - a
        bnd[c, :nseg] = loc[1:]      # ends (slot of last incidence)
        bnd[c, nseg:] = loc[:-1]     # starts (slot before first)
    return slots, bnd, needed


def _prep_core(k, x, node_idx, hedge_idx, batch):
    s = int(np.searchsorted(batch, 2 * k))
    mid = int(np.searchsorted(batch, 2 * k + 1))
    e = int(np.searchsorted(batch, 2 * k + 2))
    n0, n1 = mid - s, e - mid
    assert n0 <= GP and n1 <= GP, (n0, n1)

    sel = np.where((node_idx >= s) & (node_idx < e))[0]
    na = (node_idx[sel] - s).astype(np.int64)
    ha = hedge_idx[sel].astype(np.int64)
    col = np.where(na < n0, na, GP + na - n0)

    oA = np.argsort(ha, kind="stable")
    haS, colA = ha[oA], col[oA]
    oB = np.argsort(col, kind="stable")
    colS, haB = col[oB], ha[oB]

    deg = np.bincount(node_idx, minlength=N).astype(np.float64)[s:e]
    dinv = np.zeros(NKP, np.float32)
    loc = np.arange(e - s)
    dcols = np.where(loc < n0, loc, GP + loc - n0)
    dinv[dcols] = np.where(deg > 0, 1.0 / np.maximum(deg, 1), 0.0)

    xT = np.zeros((F_IN, NKP), np.float32)
    xT[:, dcols] = x[s:e].T

    return {
        "haS": haS, "colA": colA, "colS": colS, "haB": haB,
        "dinv": dinv, "xT": xT, "n0": n0, "n1": n1,
    }


def prepare(x, node_idx, hedge_idx, batch, W0, b0, Wc1, bc1, Wc2, bc2,
            Wg, bg, W1, b1, g1, be1, rm1, rv1, W2, b2, g2, be2, rm2, rv2, W3):
    """Compile (if needed) and build per-core input maps. Returns (nc, in_maps)."""
    global _COMPILED, _CA, _CB

    x = np.asarray(x, np.float32)
    node_idx = np.asarray(node_idx).astype(np.int64)
    hedge_idx = np.asarray(hedge_idx).astype(np.int64)
    batch_np = np.asarray(batch).astype(np.int64)

    pre = [_prep_core(k, x, node_idx, hedge_idx, batch_np) for k in range(NCORE)]

    needA = max(_chunk_lists(p["haS"], p["colA"], NSEGA, NCHA, NKP)[2]
                for p in pre)
    needB = max(_chunk_lists(p["colS"], p["haB"], NSEGB, NCHB, M_PAD)[2]
                for p in pre)
    CA = -(-needA // 16) * 16
    CB = -(-needB // 16) * 16

    if _COMPILED is None or (_CA, _CB) != (CA, CB):
        _COMPILED = _build_nc(CA, CB)
        _CA, _CB = CA, CB
    nc = _COMPILED

    # replicated (weight) inputs, with eval-BN folded into W1/W2
    k1 = np.asarray(g1) / np.sqrt(np.asarray(rv1) + EPS)
    W1f = (np.asarray(W1) * k1[None, :]).astype(np.float32)
    b1f = ((np.asarray(b1) - np.asarray(rm1)) * k1 + np.asarray(be1)).astype(np.float32)
    k2 = np.asarray(g2) / np.sqrt(np.asarray(rv2) + EPS)
    W2f = (np.asarray(W2) * k2[None, :]).astype(np.float32)
    b2f = ((np.asarray(b2) - np.asarray(rm2)) * k2 + np.asarray(be2)).astype(np.float32)

    cnt = np.bincount(hedge_idx, minlength=M_PAD).astype(np.float32)
    binv = np.where(cnt > 0, 1.0 / np.maximum(cnt, 1), 0.0).astype(np.float32)

    Wg_np = np.asarray(Wg, np.float32)
    bc1_np = np.asarray(bc1, np.float32)
    bc2_np = np.asarray(bc2, np.float32)
    shared = {
        "binv_r": np.ascontiguousarray(np.broadcast_to(binv, (128, M_PAD))),
        "W0": np.asarray(W0, np.float32), "Wc1": np.asarray(Wc1, np.float32),
        "Wc2": np.asarray(Wc2, np.float32),
        "WgA": Wg_np[:H], "WgB": Wg_np[H:],
        "W1f": W1f, "W2f": W2f, "W3": np.asarray(W3, np.float32),
        "b0c": np.asarray(b0, np.float32).reshape(-1, 1),
        "bc1c": bc1_np.reshape(-1, 1),
        "bc2c": bc2_np.reshape(-1, 1),
        "bgc": np.asarray(bg, np.float32).reshape(-1, 1),
        "b1c": b1f.reshape(-1, 1), "b2c": b2f.reshape(-1, 1),
    }

    in_maps = []
    for k in range(NCORE):
        p = pre[k]
        slA, bnA, _ = _chunk_lists(p["haS"], p["colA"], NSEGA, NCHA, NKP, CA)
        slB, bnB, _ = _chunk_lists(p["colS"], p["haB"], NSEGB, NCHB, M_PAD, CB)
        n0, n1 = p["n0"], p["n1"]
        relu1 = np.maximum(bc1_np, 0.0)
        relu2 = np.maximum(bc2_np, 0.0)
        pc = np.zeros((128, 4), np.float32)
        pc[:, 0] = relu1 * (GP - n0)
        pc[:, 1] = relu1 * (GP - n1)
        pc[:, 2] = relu2 * (GP - n0)
        pc[:, 3] = relu2 * (GP - n1)
        psc = np.zeros((128, 2), np.float32)
        psc[:, 0] = 1.0 / max(n0, 1)
        psc[:, 1] = 1.0 / max(n1, 1)
        m = {
            "xT": p["xT"],
            "idxA": _wrap_idx(slA.reshape(-1)),
            "bidxA": _wrap_idx(bnA.reshape(-1)),
            "idxB": _wrap_idx(slB.reshape(-1)),
            "bidxB": _wrap_idx(bnB.reshape(-1)),
            "dinv_r": np.ascontiguousarray(
                np.broadcast_to(p["dinv"], (128, NKP))),
            "pc": pc, "psc": psc,
        }
        m.update(shared)
        in_maps.append(m)

    return nc, in_maps


def kernel(**inputs):
    from concourse.bass_utils import run_bass_kernel_spmd

    nc, in_maps = prepare(**inputs)
    r = run_bass_kernel_spmd(nc, in_maps, core_ids=list(range(NCORE)))
    out = np.zeros((B_GRAPHS, 4), np.float32)
    for k in range(NCORE):
        o = r.results[k]["out"]
        out[2 * k] = o[:, 0]
        out[2 * k + 1] = o[:, 1]
    return out
